# revision 9
# baseline (speedup 1.0000x reference)
"""Trainium2 Bass kernel for the binary-conv BasicBlock (dense_cnn).

Computation (forward values only):
  A1   = sign(x + b11)
  out1 = x + bn1(conv3x3(A1, binw(w3)))          binw(w) = mean|w| * sign(w)
  o1   = prelu(out1 + b12, a1) + b13
  A2   = sign(o1 + b21)
  out2 = bn2(conv1x1(A2, binw(w1))) + o1
  out  = prelu(out2 + b22, a2) + b23

Strategy: data-parallel over the batch axis, 4 images per NeuronCore on 8
cores; weights/consts replicated.  Per core the 3x3 binary conv runs as 9
shifted DoubleRow fp8 matmuls over 464-column tiles spanning padded rows
1..56 only (3248 cols).  A1 = sign(x + b11) is a pure function of the
inputs, so it is packed host-side (fp8, zero halo baked in) and DMAed —
no on-device prep at all.  Element-wise work is then split DVE/ACT so
the TensorE (~27.6us/image) is the only bottleneck (GpSimd's software
tensor ops measure ~10x below roofline, so it only gets the rare b23 op):
  DVE:    (a) t = psum1*sh1 + xprep      (d) u = psum2*sh2 + p1   [in-place]
  ACT:    (b) p1 = prelu(t, a1), (c) a2 = Sign(t - tau),
          (e) out = prelu(u + K2', a2)  [contiguous; DMA does compaction]
a2 = sign(t - tau) replaces sign(prelu(t,a1)+beta) exactly for a1 > 0
(tau = -beta/a1 if beta >= 0 else -beta).  conv2 matmuls of image i are
interleaved into image i+1's conv1 PE stream via a pending queue so the
PE never idles on drains; image 3's tail is drained at quarter-image
granularity.  Weights load on the ACT DMA ring and outputs leave on the
GpSimd ring so they never queue behind activation inputs.
"""

import numpy as np
import ml_dtypes
from collections import deque

C = 256
H = W = 56
PH = 58                    # padded image side
NPIX = PH * PH             # 3364
TW = 464                   # matmul tile width (8 padded rows)
NT = 7                     # tiles per image
TOT = NT * TW              # 3248 = 56 rows * 58 cols  (padded rows 1..56)
G0 = PH                    # first computed pixel (row 1, col 0) in A1 coords
HALO = 16
A1W = 3408                 # 16 + NPIX + 16 -> next multiple of 16
BPC = 4                    # images per core
NCORES = 8
EPS = 1e-5

_CACHE = {}
_FLAGS = {"has_b23": False}


def _split_drain_waits(m, max_waits=1):
    """This toolchain's walrus rejects instructions carrying more than ~1-2
    sync waits; hoist extra waits onto preceding single-wait EventSemaphore
    ops on the same engine (semantically identical: the engine blocks on
    each wait in sequence before executing the instruction)."""
    import copy as _copy
    from concourse import mybir

    new_module = _copy.replace(m, functions=[])
    for function in m.functions:
        new_function = _copy.replace(function, blocks=[])
        new_function.set_allocations_from_list(function.allocations)
        for block in function.blocks:
            out = []
            for inst in block.instructions:
                si = inst.sync_info
                if si is not None and len(si.on_wait) > max_waits:
                    waits = list(si.on_wait)
                    keep = waits[:max_waits] if not isinstance(
                        inst, mybir.InstDrain) else []
                    hoist = waits[len(keep):]
                    for i, wt in enumerate(hoist):
                        out.append(
                            mybir.InstEventSemaphore(
                                name=f"{inst.name}-wsplit{i}",
                                opcode="EventSemaphore",
                                engine=inst.engine,
                                sync_info=mybir.SyncInfo(on_wait=[wt], on_update=[]),
                            )
                        )
                    inst.sync_info = mybir.SyncInfo(
                        on_wait=keep, on_update=list(si.on_update)
                    )
                out.append(inst)
            new_block = _copy.replace(block, instructions=out)
            new_function.blocks.append(new_block)
        new_module.functions.append(new_function)
    return new_module


def build_nc(has_b23=None):
    """Build (once per structure flag) the per-core Bass program."""
    if has_b23 is None:
        has_b23 = _FLAGS["has_b23"]
    key = ("nc", has_b23)
    if key in _CACHE:
        return _CACHE[key]
    import concourse.bass as bass
    import concourse.tile as tile
    from concourse import mybir

    Alu = mybir.AluOpType
    AF = mybir.ActivationFunctionType
    f32 = mybir.dt.float32
    bf16 = mybir.dt.bfloat16
    fp8 = mybir.dt.float8e4
    DR = mybir.MatmulPerfMode.DoubleRow

    nc = bass.Bass(trn_type="TRN2", debug=False)
    # xprep = x + K1, stored for padded rows 1..56 (58 cols each, col 0/57
    # are zero pads): dram index = (row-1)*58 + col
    x_d = nc.dram_tensor("xprep", [BPC, 2, 128, TOT], f32, kind="ExternalInput")
    # A1 = sign(x + b11), host-packed fp8 in halo layout (16 + 3364 + 16)
    a_d = nc.dram_tensor("a1h", [BPC, 2, 128, A1W], fp8, kind="ExternalInput")
    w3_d = nc.dram_tensor("w3f", [128, 9 * 2 * 2 * 128], fp8, kind="ExternalInput")
    w1_d = nc.dram_tensor("w1f", [128, 2 * 2 * 128], fp8, kind="ExternalInput")
    c_d = nc.dram_tensor("consts", [2, 128, 8], f32, kind="ExternalInput")
    o_d = nc.dram_tensor("out", [BPC, 2, 128, H * W], f32, kind="ExternalOutput")

    with tile.TileContext(nc) as tc:
        with (
            tc.tile_pool(name="wpool", bufs=1) as wpool,
            tc.tile_pool(name="xpool", bufs=2) as xpool,
            tc.tile_pool(name="a1pool", bufs=2) as a1pool,
            tc.tile_pool(name="tpool", bufs=2) as tpool,
            tc.tile_pool(name="p1pool", bufs=2) as p1pool,
            tc.tile_pool(name="a2pool", bufs=2) as a2pool,
            tc.tile_pool(name="opool", bufs=1) as opool,
            tc.tile_pool(name="ps1", bufs=4, space="PSUM") as ps1p,
            tc.tile_pool(name="ps2", bufs=4, space="PSUM") as ps2p,
        ):
            # ---- constants / weights (resident); weights ride the Vector
            # DMA ring so the Sync ring starts on img0 activations at once
            csb = []
            for kc in range(2):
                ct = wpool.tile([128, 8], f32, tag=f"c_{kc}")
                nc.sync.dma_start(ct[:], c_d.ap()[kc])
                csb.append(ct)
            w3sb = wpool.tile([128, 9 * 2 * 2 * 128], fp8, tag="w3")
            nc.scalar.dma_start(w3sb[:], w3_d.ap())
            w1sb = wpool.tile([128, 2 * 2 * 128], fp8, tag="w1")
            nc.scalar.dma_start(w1sb[:], w1_d.ap())
            w3v = w3sb[:].rearrange("p (g two m) -> p g two m", two=2, m=128)
            w1v = w1sb[:].rearrange("p (g two m) -> p g two m", two=2, m=128)

            def cc(kc, j):
                return csb[kc][:, j : j + 1]

            # const j-layout: 0 unused, 1 -tau, 2 K2', 3 a1, 4 a2, 5 b23,
            # 6 sh1, 7 sh2

            # ---- per-image state ----
            xm = [None] * BPC   # xprep tiles: [kc]
            a1m = [None] * BPC  # A1 tiles (host-packed)
            tbs = [None] * BPC  # t tiles per mc (bf16)
            pbs = [None] * BPC  # p1/u tiles per mc (bf16)
            a2s = [None] * BPC  # a2 tiles [128, 2, TOT] fp8
            outs = [None] * BPC  # padded out tiles [128, TOT] f32 per mc
            pending = deque()   # conv2 closures: one MM2 + one (d) each

            def alloc_img(img):
                xm[img] = [
                    xpool.tile([128, TOT], f32, tag=f"xk_{kc}", name=f"xk_{kc}")
                    for kc in range(2)
                ]
                a1m[img] = a1pool.tile([128, 2 * A1W], fp8, tag="a1m", name="a1m")

            def dma_a1(img):
                for kc in range(2):
                    nc.sync.dma_start(
                        a1m[img][:, kc * A1W : (kc + 1) * A1W], a_d.ap()[img, kc]
                    )

            def dma_x(img, kc, chunks=1):
                n = TOT // chunks
                for ci in range(chunks):
                    nc.sync.dma_start(
                        xm[img][kc][:, ci * n : (ci + 1) * n],
                        x_d.ap()[img, kc][:, ci * n : (ci + 1) * n],
                    )

            def a1_rhs(img, t, kh, kw):
                v = a1m[img][:].rearrange("p (two w) -> p two w", two=2)
                off = HALO + G0 + TW * t + (kh - 1) * PH + (kw - 1)
                return v[:, :, off : off + TW]

            def emit_b(img, lo, hi):
                # p1 = prelu(t, a1) on ACT
                for mc in range(2):
                    nc.scalar.activation(
                        pbs[img][mc][:, lo:hi], tbs[img][mc][:, lo:hi],
                        AF.Prelu, alpha=cc(mc, 3),
                    )

            def emit_c(img, lo, hi):
                # a2 = Sign(t - tau) on ACT (bias slot carries -tau)
                for mc in range(2):
                    nc.scalar.activation(
                        a2s[img][:, mc, lo:hi], tbs[img][mc][:, lo:hi],
                        AF.Sign, bias=cc(mc, 1),
                    )

            def emit_eg(img, h0, h1):
                # (e) out = prelu(u + K2', a2), contiguous in padded coords;
                # the output DMA compacts 58 -> 56 cols (on the GpSimd ring)
                for mc in range(2):
                    ot = outs[img][mc]
                    nc.scalar.activation(
                        ot[:, h0 * PH : h1 * PH],
                        pbs[img][mc][:, h0 * PH : h1 * PH],
                        AF.Prelu, bias=cc(mc, 2), alpha=cc(mc, 4),
                    )
                    if has_b23:
                        nc.gpsimd.tensor_scalar(
                            ot[:, h0 * PH : h1 * PH], ot[:, h0 * PH : h1 * PH],
                            cc(mc, 5), None, Alu.add,
                        )
                    src = ot[:].rearrange("p (h w) -> p h w", w=PH)[
                        :, h0:h1, 1:57
                    ]
                    nc.gpsimd.dma_start(
                        o_d.ap()[img, mc][:, h0 * W : h1 * W], src
                    )

            def queue_conv2(img):
                outs[img] = [
                    opool.tile([128, TOT], f32, tag=f"o_{mc}", name=f"o_{mc}")
                    for mc in range(2)
                ]

                def mk(t, mc):
                    def emit():
                        ps = ps2p.tile([128, 512], f32, tag="ps2", name="ps2")
                        nc.tensor.matmul(
                            ps[:, :TW], w1v[:, mc],
                            a2s[img][:, :, TW * t : TW * (t + 1)],
                            start=True, stop=True, perf_mode=DR,
                        )
                        sl = pbs[img][mc][:, TW * t : TW * (t + 1)]
                        # (d): u = psum2*sh2 + p1, in place over p1
                        nc.vector.scalar_tensor_tensor(
                            sl, ps[:, :TW], cc(mc, 7), sl, Alu.mult, Alu.add
                        )
                    return emit

                for t in range(NT):
                    for mc in range(2):
                        pending.append(mk(t, mc))

            def conv1(img):
                last = img == BPC - 1
                tbs[img] = [
                    tpool.tile([128, TOT], bf16, tag=f"t_{mc}", name=f"t_{mc}")
                    for mc in range(2)
                ]
                pbs[img] = [
                    p1pool.tile([128, TOT], bf16, tag=f"p1_{mc}", name=f"p1_{mc}")
                    for mc in range(2)
                ]
                a2s[img] = a2pool.tile([128, 2, TOT], fp8, tag="a2", name="a2")
                for t in range(NT):
                    for mc in range(2):
                        ps = ps1p.tile([128, 512], f32, tag="ps1", name="ps1")
                        for sh in range(9):
                            kh, kw = divmod(sh, 3)
                            nc.tensor.matmul(
                                ps[:, :TW], w3v[:, sh * 2 + mc],
                                a1_rhs(img, t, kh, kw),
                                start=(sh == 0), stop=(sh == 8), perf_mode=DR,
                            )
                        # (a): t = psum1*sh1 + xprep
                        nc.vector.scalar_tensor_tensor(
                            tbs[img][mc][:, TW * t : TW * (t + 1)],
                            ps[:, :TW], cc(mc, 6),
                            xm[img][mc][:, TW * t : TW * (t + 1)],
                            Alu.mult, Alu.add,
                        )
                        if pending:
                            pending.popleft()()
                    # hooks
                    if t == 0 and not last:
                        alloc_img(img + 1)
                        dma_a1(img + 1)
                        dma_x(img + 1, 0)
                    elif t == 1 and last:
                        emit_c(img, 0, 2 * TW)
                    elif t == 2 and not last:
                        dma_x(img + 1, 1)
                    elif t == 3:
                        if last:
                            emit_c(img, 2 * TW, 4 * TW)
                        else:
                            emit_c(img, 0, 4 * TW)
                        emit_b(img, 0, 4 * TW)
                    elif t == 4 and img > 0:
                        emit_eg(img - 1, 0, 28)
                    elif t == 5 and last:
                        emit_c(img, 4 * TW, 6 * TW)
                # post-loop
                if last:
                    emit_c(img, 6 * TW, TOT)
                else:
                    emit_c(img, 4 * TW, TOT)
                emit_b(img, 4 * TW, TOT)
                if img > 0:
                    emit_eg(img - 1, 28, 56)

            # ---- main sequence ----
            alloc_img(0)
            dma_a1(0)
            dma_x(0, 0, chunks=4)
            dma_x(0, 1, chunks=4)
            for img in range(BPC):
                conv1(img)
                queue_conv2(img)
            # tail: drain image 3's conv2 at quarter-image granularity
            li = BPC - 1
            for qi, (h0, h1) in enumerate(((0, 14), (14, 28), (28, 42), (42, 56))):
                npop = 4 if qi < 3 else 2
                for _ in range(npop):
                    if pending:
                        pending.popleft()()
                emit_eg(li, h0, h1)

    _CACHE[key] = nc
    return nc


def _host_fold(w3, w1, b11, b12, b13, b21, b22, b23,
               g1, be1, m1, v1, g2, be2, m2, v2, a1, a2):
    f = np.float32
    s3 = np.mean(np.abs(w3), axis=(1, 2, 3)).astype(f)
    s1 = np.mean(np.abs(w1), axis=(1, 2, 3)).astype(f)
    inv1 = (g1 / np.sqrt(v1 + EPS)).astype(f)
    inv2 = (g2 / np.sqrt(v2 + EPS)).astype(f)
    sh1 = s3 * inv1
    ch1 = be1 - m1 * inv1
    sh2 = s1 * inv2
    ch2 = be2 - m2 * inv2
    K1 = (ch1 + b12).astype(f)
    beta = (b13 + b21).astype(f)
    # a2 = sign(prelu(t, a1) + beta) = sign(t - tau) for a1 > 0
    tau = np.where(beta >= 0, -beta / a1, -beta).astype(f)
    K2p = (ch2 + b13 + b22).astype(f)

    fp8 = ml_dtypes.float8_e4m3
    # DoubleRow lhsT layout: [k, ((sh*2+mc)*2+i)*128+m] with i the K-half
    W3 = np.sign(w3).astype(fp8)                                # [O, I, 3, 3]
    W3 = W3.reshape(2, 128, 2, 128, 3, 3)                       # [mc, m, i, k, kh, kw]
    W3 = W3.transpose(3, 4, 5, 0, 2, 1)                         # [k, kh, kw, mc, i, m]
    W3f = np.ascontiguousarray(W3.reshape(128, 9 * 2 * 2 * 128))
    W1 = np.sign(w1).astype(fp8)                                # [O, I, 1, 1]
    W1 = W1.reshape(2, 128, 2, 128)                             # [mc, m, i, k]
    W1 = W1.transpose(3, 0, 2, 1)                               # [k, mc, i, m]
    W1f = np.ascontiguousarray(W1.reshape(128, 2 * 2 * 128))

    consts = np.zeros((2, 128, 8), f)
    for kc in range(2):
        sl = slice(kc * 128, (kc + 1) * 128)
        consts[kc, :, 1] = -tau[sl]
        consts[kc, :, 2] = K2p[sl]
        consts[kc, :, 3] = a1[sl]
        consts[kc, :, 4] = a2[sl]
        consts[kc, :, 5] = b23[sl]
        consts[kc, :, 6] = sh1[sl]
        consts[kc, :, 7] = sh2[sl]
    ok = bool((a1 > 0).all()) and bool(np.isfinite(consts).all())
    return W3f, W1f, consts, K1, ok, bool(np.any(b23 != 0))


def _run(in_maps, trace=False, tmpdir=None, trace_kwargs={}):
    from concourse import bass_utils

    nc = build_nc()
    skey = ("split", _FLAGS["has_b23"])
    if not _CACHE.get(skey):
        # walrus workaround applied only for the HW path (CoreSim rejects
        # post-scheduling instruction edits)
        nc.m = _split_drain_waits(nc.m)
        _CACHE[skey] = True
    return bass_utils.run_bass_kernel_spmd(
        nc,
        in_maps,
        core_ids=list(range(NCORES)),
        trace=trace,
        tmpdir=tmpdir,
        trace_kwargs=trace_kwargs,
    )


def make_in_maps(x, w3, w1, **params):
    x = np.asarray(x, np.float32)
    b11 = np.asarray(params["b11"], np.float32)
    W3f, W1f, consts, K1, ok, has_b23 = _host_fold(
        np.asarray(w3, np.float32), np.asarray(w1, np.float32),
        **{k: np.asarray(v, np.float32) for k, v in params.items()})
    _FLAGS["has_b23"] = has_b23
    _FLAGS["ok"] = ok
    xp = np.zeros((x.shape[0], C, H, PH), np.float32)
    xp[:, :, :, 1:57] = x + K1[None, :, None, None]
    x_prep = xp.reshape(NCORES, BPC, 2, 128, TOT)
    # host-packed A1 = sign(x + b11), fp8, zero halo/pads baked in
    ap8 = np.zeros((x.shape[0], C, A1W), ml_dtypes.float8_e4m3)
    s = np.sign(x + b11[None, :, None, None]).astype(ml_dtypes.float8_e4m3)
    av = ap8[:, :, HALO : HALO + NPIX].reshape(x.shape[0], C, PH, PH)
    av[:, :, 1:57, 1:57] = s
    a1h = ap8.reshape(NCORES, BPC, 2, 128, A1W)
    return [
        {"xprep": np.ascontiguousarray(x_prep[c]),
         "a1h": np.ascontiguousarray(a1h[c]),
         "w3f": W3f, "w1f": W1f, "consts": consts}
        for c in range(NCORES)
    ]


def assemble_out(results):
    outs = [results[c]["out"].reshape(BPC, C, H, W) for c in range(NCORES)]
    return np.ascontiguousarray(
        np.concatenate(outs, axis=0).astype(np.float32)
    )


def _fallback_numpy(x, w3, w1, b11, b12, b13, b21, b22, b23,
                    g1, be1, m1, v1, g2, be2, m2, v2, a1, a2):
    # Straightforward reference math in numpy; only used if an assumption of
    # the device kernel (a1 > 0, finite folded consts) is violated.
    def cb(p):
        return p[None, :, None, None]

    def conv_np(a, w, pad):
        N, Ci, Hh, Ww = a.shape
        O, I, kh, kw = w.shape
        ap = np.pad(a, ((0, 0), (0, 0), (pad, pad), (pad, pad)))
        out = np.zeros((N, O, Hh, Ww), np.float32)
        wm = w.reshape(O, -1)
        for n in range(N):
            cols = np.empty((I * kh * kw, Hh * Ww), np.float32)
            idx = 0
            for i in range(I):
                for dh in range(kh):
                    for dw in range(kw):
                        cols[idx] = ap[n, i, dh : dh + Hh, dw : dw + Ww].ravel()
                        idx += 1
            out[n] = (wm @ cols).reshape(O, Hh, Ww)
        return out

    def bn(t, g, b, mm, v):
        inv = g / np.sqrt(v + EPS)
        return t * cb(inv) + cb(b - mm * inv)

    def prelu(t, a):
        return np.where(t > 0, t, cb(a) * t)

    s3 = np.mean(np.abs(w3), axis=(1, 2, 3), keepdims=True)
    s1 = np.mean(np.abs(w1), axis=(1, 2, 3), keepdims=True)
    o1 = conv_np(np.sign(x + cb(b11)), np.sign(w3) * s3, 1)
    o1 = x + bn(o1, g1, be1, m1, v1)
    o1 = prelu(o1 + cb(b12), a1) + cb(b13)
    o2 = conv_np(np.sign(o1 + cb(b21)), np.sign(w1) * s1, 0)
    o2 = bn(o2, g2, be2, m2, v2) + o1
    o2 = prelu(o2 + cb(b22), a2) + cb(b23)
    return o2.astype(np.float32)


def kernel(**inputs):
    inputs = {k: np.asarray(v) for k, v in inputs.items()}
    in_maps = make_in_maps(**inputs)
    if not _FLAGS.get("ok", True):
        return _fallback_numpy(**{k: np.asarray(v, np.float32)
                                  for k, v in inputs.items()})
    res = _run(in_maps, trace=False)
    return assemble_out(res.results)


# revision 10
# speedup vs baseline: 1.0342x; 1.0342x over previous
"""Trainium2 Bass kernel for the binary-conv BasicBlock (dense_cnn).

Computation (forward values only):
  A1   = sign(x + b11)
  out1 = x + bn1(conv3x3(A1, binw(w3)))          binw(w) = mean|w| * sign(w)
  o1   = prelu(out1 + b12, a1) + b13
  A2   = sign(o1 + b21)
  out2 = bn2(conv1x1(A2, binw(w1))) + o1
  out  = prelu(out2 + b22, a2) + b23

Strategy: data-parallel over the batch axis, 4 images per NeuronCore on 8
cores; weights/consts replicated.  Per core the 3x3 binary conv runs as 9
shifted DoubleRow fp8 matmuls over 464-column tiles spanning padded rows
1..56 only (3248 cols).  A1 = sign(x + b11) is a pure function of the
inputs, so it is packed host-side (fp8, zero halo baked in) and DMAed —
no on-device prep at all.  Element-wise work is then split DVE/ACT so
the TensorE (~27.6us/image) is the only bottleneck (GpSimd's software
tensor ops measure ~10x below roofline, so it only gets the rare b23 op):
  DVE:    (a) t = psum1*sh1 + xprep      (d) u = psum2*sh2 + p1   [in-place]
  ACT:    (b) p1 = prelu(t, a1), (c) a2 = Sign(t - tau),
          (e) out = prelu(u + K2', a2)  [contiguous; DMA does compaction]
a2 = sign(t - tau) replaces sign(prelu(t,a1)+beta) exactly for a1 > 0
(tau = -beta/a1 if beta >= 0 else -beta).  conv2 matmuls of image i are
interleaved into image i+1's conv1 PE stream via a pending queue so the
PE never idles on drains; image 3's tail is drained at quarter-image
granularity.  Weights load on the ACT DMA ring (the Sync ring starts on
img0 activations at once); outputs share the Sync HW ring — the GpSimd
ring is software-DGE and far too slow for bulk traffic.
"""

import numpy as np
import ml_dtypes
from collections import deque

C = 256
H = W = 56
PH = 58                    # padded image side
NPIX = PH * PH             # 3364
TW = 464                   # matmul tile width (8 padded rows)
NT = 7                     # tiles per image
TOT = NT * TW              # 3248 = 56 rows * 58 cols  (padded rows 1..56)
G0 = PH                    # first computed pixel (row 1, col 0) in A1 coords
HALO = 16
A1W = 3408                 # 16 + NPIX + 16 -> next multiple of 16
BPC = 4                    # images per core
NCORES = 8
EPS = 1e-5

_CACHE = {}
_FLAGS = {"has_b23": False}


def _split_drain_waits(m, max_waits=1):
    """This toolchain's walrus rejects instructions carrying more than ~1-2
    sync waits; hoist extra waits onto preceding single-wait EventSemaphore
    ops on the same engine (semantically identical: the engine blocks on
    each wait in sequence before executing the instruction)."""
    import copy as _copy
    from concourse import mybir

    new_module = _copy.replace(m, functions=[])
    for function in m.functions:
        new_function = _copy.replace(function, blocks=[])
        new_function.set_allocations_from_list(function.allocations)
        for block in function.blocks:
            out = []
            for inst in block.instructions:
                si = inst.sync_info
                if si is not None and len(si.on_wait) > max_waits:
                    waits = list(si.on_wait)
                    keep = waits[:max_waits] if not isinstance(
                        inst, mybir.InstDrain) else []
                    hoist = waits[len(keep):]
                    for i, wt in enumerate(hoist):
                        out.append(
                            mybir.InstEventSemaphore(
                                name=f"{inst.name}-wsplit{i}",
                                opcode="EventSemaphore",
                                engine=inst.engine,
                                sync_info=mybir.SyncInfo(on_wait=[wt], on_update=[]),
                            )
                        )
                    inst.sync_info = mybir.SyncInfo(
                        on_wait=keep, on_update=list(si.on_update)
                    )
                out.append(inst)
            new_block = _copy.replace(block, instructions=out)
            new_function.blocks.append(new_block)
        new_module.functions.append(new_function)
    return new_module


def build_nc(has_b23=None):
    """Build (once per structure flag) the per-core Bass program."""
    if has_b23 is None:
        has_b23 = _FLAGS["has_b23"]
    key = ("nc", has_b23)
    if key in _CACHE:
        return _CACHE[key]
    import concourse.bass as bass
    import concourse.tile as tile
    from concourse import mybir

    Alu = mybir.AluOpType
    AF = mybir.ActivationFunctionType
    f32 = mybir.dt.float32
    bf16 = mybir.dt.bfloat16
    fp8 = mybir.dt.float8e4
    DR = mybir.MatmulPerfMode.DoubleRow

    nc = bass.Bass(trn_type="TRN2", debug=False)
    # xprep = x + K1, stored for padded rows 1..56 (58 cols each, col 0/57
    # are zero pads): dram index = (row-1)*58 + col
    x_d = nc.dram_tensor("xprep", [BPC, 2, 128, TOT], f32, kind="ExternalInput")
    # A1 = sign(x + b11), host-packed fp8 in halo layout (16 + 3364 + 16)
    a_d = nc.dram_tensor("a1h", [BPC, 2, 128, A1W], fp8, kind="ExternalInput")
    w3_d = nc.dram_tensor("w3f", [128, 9 * 2 * 2 * 128], fp8, kind="ExternalInput")
    w1_d = nc.dram_tensor("w1f", [128, 2 * 2 * 128], fp8, kind="ExternalInput")
    c_d = nc.dram_tensor("consts", [2, 128, 8], f32, kind="ExternalInput")
    o_d = nc.dram_tensor("out", [BPC, 2, 128, H * W], f32, kind="ExternalOutput")

    with tile.TileContext(nc) as tc:
        with (
            tc.tile_pool(name="wpool", bufs=1) as wpool,
            tc.tile_pool(name="xpool", bufs=2) as xpool,
            tc.tile_pool(name="a1pool", bufs=2) as a1pool,
            tc.tile_pool(name="tpool", bufs=2) as tpool,
            tc.tile_pool(name="p1pool", bufs=2) as p1pool,
            tc.tile_pool(name="a2pool", bufs=2) as a2pool,
            tc.tile_pool(name="opool", bufs=2) as opool,
            tc.tile_pool(name="ps1", bufs=4, space="PSUM") as ps1p,
            tc.tile_pool(name="ps2", bufs=4, space="PSUM") as ps2p,
        ):
            # ---- constants / weights (resident); weights ride the Vector
            # DMA ring so the Sync ring starts on img0 activations at once
            csb = []
            for kc in range(2):
                ct = wpool.tile([128, 8], f32, tag=f"c_{kc}")
                nc.sync.dma_start(ct[:], c_d.ap()[kc])
                csb.append(ct)
            w3sb = wpool.tile([128, 9 * 2 * 2 * 128], fp8, tag="w3")
            nc.scalar.dma_start(w3sb[:], w3_d.ap())
            w1sb = wpool.tile([128, 2 * 2 * 128], fp8, tag="w1")
            nc.scalar.dma_start(w1sb[:], w1_d.ap())
            w3v = w3sb[:].rearrange("p (g two m) -> p g two m", two=2, m=128)
            w1v = w1sb[:].rearrange("p (g two m) -> p g two m", two=2, m=128)

            def cc(kc, j):
                return csb[kc][:, j : j + 1]

            # const j-layout: 0 unused, 1 -tau, 2 K2', 3 a1, 4 a2, 5 b23,
            # 6 sh1, 7 sh2

            # ---- per-image state ----
            xm = [None] * BPC   # xprep tiles: [kc]
            a1m = [None] * BPC  # A1 tiles (host-packed)
            tbs = [None] * BPC  # t tiles per mc (bf16)
            pbs = [None] * BPC  # p1/u tiles per mc (bf16)
            a2s = [None] * BPC  # a2 tiles [128, 2, TOT] fp8
            outs = [None] * BPC  # padded out tiles [128, TOT] f32 per mc
            pending = deque()   # conv2 closures: one MM2 + one (d) each

            def alloc_img(img):
                xm[img] = [
                    xpool.tile([128, TOT], f32, tag=f"xk_{kc}", name=f"xk_{kc}")
                    for kc in range(2)
                ]
                a1m[img] = a1pool.tile([128, 2 * A1W], fp8, tag="a1m", name="a1m")

            def dma_a1(img):
                for kc in range(2):
                    nc.sync.dma_start(
                        a1m[img][:, kc * A1W : (kc + 1) * A1W], a_d.ap()[img, kc]
                    )

            def dma_x(img, kc, chunks=1):
                n = TOT // chunks
                for ci in range(chunks):
                    nc.sync.dma_start(
                        xm[img][kc][:, ci * n : (ci + 1) * n],
                        x_d.ap()[img, kc][:, ci * n : (ci + 1) * n],
                    )

            def a1_rhs(img, t, kh, kw):
                v = a1m[img][:].rearrange("p (two w) -> p two w", two=2)
                off = HALO + G0 + TW * t + (kh - 1) * PH + (kw - 1)
                return v[:, :, off : off + TW]

            def emit_b(img, lo, hi):
                # p1 = prelu(t, a1) on ACT
                for mc in range(2):
                    nc.scalar.activation(
                        pbs[img][mc][:, lo:hi], tbs[img][mc][:, lo:hi],
                        AF.Prelu, alpha=cc(mc, 3),
                    )

            def emit_c(img, lo, hi):
                # a2 = Sign(t - tau) on ACT (bias slot carries -tau)
                for mc in range(2):
                    nc.scalar.activation(
                        a2s[img][:, mc, lo:hi], tbs[img][mc][:, lo:hi],
                        AF.Sign, bias=cc(mc, 1),
                    )

            def emit_eg(img, h0, h1):
                # (e) out = prelu(u + K2', a2), contiguous in padded coords;
                # the output DMA compacts 58 -> 56 cols (on the GpSimd ring)
                for mc in range(2):
                    ot = outs[img][mc]
                    nc.scalar.activation(
                        ot[:, h0 * PH : h1 * PH],
                        pbs[img][mc][:, h0 * PH : h1 * PH],
                        AF.Prelu, bias=cc(mc, 2), alpha=cc(mc, 4),
                    )
                    if has_b23:
                        nc.gpsimd.tensor_scalar(
                            ot[:, h0 * PH : h1 * PH], ot[:, h0 * PH : h1 * PH],
                            cc(mc, 5), None, Alu.add,
                        )
                    src = ot[:].rearrange("p (h w) -> p h w", w=PH)[
                        :, h0:h1, 1:57
                    ]
                    nc.sync.dma_start(
                        o_d.ap()[img, mc][:, h0 * W : h1 * W], src
                    )

            def queue_conv2(img):
                outs[img] = [
                    opool.tile([128, TOT], f32, tag=f"o_{mc}", name=f"o_{mc}")
                    for mc in range(2)
                ]

                def mk(t, mc):
                    def emit():
                        ps = ps2p.tile([128, 512], f32, tag="ps2", name="ps2")
                        nc.tensor.matmul(
                            ps[:, :TW], w1v[:, mc],
                            a2s[img][:, :, TW * t : TW * (t + 1)],
                            start=True, stop=True, perf_mode=DR,
                        )
                        sl = pbs[img][mc][:, TW * t : TW * (t + 1)]
                        # (d): u = psum2*sh2 + p1, in place over p1
                        nc.vector.scalar_tensor_tensor(
                            sl, ps[:, :TW], cc(mc, 7), sl, Alu.mult, Alu.add
                        )
                    return emit

                for t in range(NT):
                    for mc in range(2):
                        pending.append(mk(t, mc))

            def conv1(img):
                last = img == BPC - 1
                tbs[img] = [
                    tpool.tile([128, TOT], bf16, tag=f"t_{mc}", name=f"t_{mc}")
                    for mc in range(2)
                ]
                pbs[img] = [
                    p1pool.tile([128, TOT], bf16, tag=f"p1_{mc}", name=f"p1_{mc}")
                    for mc in range(2)
                ]
                a2s[img] = a2pool.tile([128, 2, TOT], fp8, tag="a2", name="a2")
                for t in range(NT):
                    for mc in range(2):
                        ps = ps1p.tile([128, 512], f32, tag="ps1", name="ps1")
                        for sh in range(9):
                            kh, kw = divmod(sh, 3)
                            nc.tensor.matmul(
                                ps[:, :TW], w3v[:, sh * 2 + mc],
                                a1_rhs(img, t, kh, kw),
                                start=(sh == 0), stop=(sh == 8), perf_mode=DR,
                            )
                        # (a): t = psum1*sh1 + xprep
                        nc.vector.scalar_tensor_tensor(
                            tbs[img][mc][:, TW * t : TW * (t + 1)],
                            ps[:, :TW], cc(mc, 6),
                            xm[img][mc][:, TW * t : TW * (t + 1)],
                            Alu.mult, Alu.add,
                        )
                        if pending:
                            pending.popleft()()
                    # hooks
                    if t == 0 and not last:
                        alloc_img(img + 1)
                        dma_a1(img + 1)
                        dma_x(img + 1, 0)
                    elif t == 1 and last:
                        emit_c(img, 0, 2 * TW)
                    elif t == 2 and not last:
                        dma_x(img + 1, 1)
                    elif t == 3:
                        if last:
                            emit_c(img, 2 * TW, 4 * TW)
                        else:
                            emit_c(img, 0, 4 * TW)
                        emit_b(img, 0, 4 * TW)
                    elif t == 4 and img > 0:
                        emit_eg(img - 1, 0, 28)
                    elif t == 5 and last:
                        emit_c(img, 4 * TW, 6 * TW)
                # post-loop
                if last:
                    emit_c(img, 6 * TW, TOT)
                else:
                    emit_c(img, 4 * TW, TOT)
                emit_b(img, 4 * TW, TOT)
                if img > 0:
                    emit_eg(img - 1, 28, 56)

            # ---- main sequence ----
            alloc_img(0)
            dma_a1(0)
            dma_x(0, 0, chunks=4)
            dma_x(0, 1, chunks=4)
            for img in range(BPC):
                conv1(img)
                queue_conv2(img)
            # tail: drain image 3's conv2 at quarter-image granularity
            li = BPC - 1
            for qi, (h0, h1) in enumerate(((0, 14), (14, 28), (28, 42), (42, 56))):
                npop = 4 if qi < 3 else 2
                for _ in range(npop):
                    if pending:
                        pending.popleft()()
                emit_eg(li, h0, h1)

    _CACHE[key] = nc
    return nc


def _host_fold(w3, w1, b11, b12, b13, b21, b22, b23,
               g1, be1, m1, v1, g2, be2, m2, v2, a1, a2):
    f = np.float32
    s3 = np.mean(np.abs(w3), axis=(1, 2, 3)).astype(f)
    s1 = np.mean(np.abs(w1), axis=(1, 2, 3)).astype(f)
    inv1 = (g1 / np.sqrt(v1 + EPS)).astype(f)
    inv2 = (g2 / np.sqrt(v2 + EPS)).astype(f)
    sh1 = s3 * inv1
    ch1 = be1 - m1 * inv1
    sh2 = s1 * inv2
    ch2 = be2 - m2 * inv2
    K1 = (ch1 + b12).astype(f)
    beta = (b13 + b21).astype(f)
    # a2 = sign(prelu(t, a1) + beta) = sign(t - tau) for a1 > 0
    tau = np.where(beta >= 0, -beta / a1, -beta).astype(f)
    K2p = (ch2 + b13 + b22).astype(f)

    fp8 = ml_dtypes.float8_e4m3
    # DoubleRow lhsT layout: [k, ((sh*2+mc)*2+i)*128+m] with i the K-half
    W3 = np.sign(w3).astype(fp8)                                # [O, I, 3, 3]
    W3 = W3.reshape(2, 128, 2, 128, 3, 3)                       # [mc, m, i, k, kh, kw]
    W3 = W3.transpose(3, 4, 5, 0, 2, 1)                         # [k, kh, kw, mc, i, m]
    W3f = np.ascontiguousarray(W3.reshape(128, 9 * 2 * 2 * 128))
    W1 = np.sign(w1).astype(fp8)                                # [O, I, 1, 1]
    W1 = W1.reshape(2, 128, 2, 128)                             # [mc, m, i, k]
    W1 = W1.transpose(3, 0, 2, 1)                               # [k, mc, i, m]
    W1f = np.ascontiguousarray(W1.reshape(128, 2 * 2 * 128))

    consts = np.zeros((2, 128, 8), f)
    for kc in range(2):
        sl = slice(kc * 128, (kc + 1) * 128)
        consts[kc, :, 1] = -tau[sl]
        consts[kc, :, 2] = K2p[sl]
        consts[kc, :, 3] = a1[sl]
        consts[kc, :, 4] = a2[sl]
        consts[kc, :, 5] = b23[sl]
        consts[kc, :, 6] = sh1[sl]
        consts[kc, :, 7] = sh2[sl]
    ok = bool((a1 > 0).all()) and bool(np.isfinite(consts).all())
    return W3f, W1f, consts, K1, ok, bool(np.any(b23 != 0))


def _run(in_maps, trace=False, tmpdir=None, trace_kwargs={}):
    from concourse import bass_utils

    nc = build_nc()
    skey = ("split", _FLAGS["has_b23"])
    if not _CACHE.get(skey):
        # walrus workaround applied only for the HW path (CoreSim rejects
        # post-scheduling instruction edits)
        nc.m = _split_drain_waits(nc.m)
        _CACHE[skey] = True
    return bass_utils.run_bass_kernel_spmd(
        nc,
        in_maps,
        core_ids=list(range(NCORES)),
        trace=trace,
        tmpdir=tmpdir,
        trace_kwargs=trace_kwargs,
    )


def make_in_maps(x, w3, w1, **params):
    x = np.asarray(x, np.float32)
    b11 = np.asarray(params["b11"], np.float32)
    W3f, W1f, consts, K1, ok, has_b23 = _host_fold(
        np.asarray(w3, np.float32), np.asarray(w1, np.float32),
        **{k: np.asarray(v, np.float32) for k, v in params.items()})
    _FLAGS["has_b23"] = has_b23
    _FLAGS["ok"] = ok
    xp = np.zeros((x.shape[0], C, H, PH), np.float32)
    xp[:, :, :, 1:57] = x + K1[None, :, None, None]
    x_prep = xp.reshape(NCORES, BPC, 2, 128, TOT)
    # host-packed A1 = sign(x + b11), fp8, zero halo/pads baked in
    ap8 = np.zeros((x.shape[0], C, A1W), ml_dtypes.float8_e4m3)
    s = np.sign(x + b11[None, :, None, None]).astype(ml_dtypes.float8_e4m3)
    av = ap8[:, :, HALO : HALO + NPIX].reshape(x.shape[0], C, PH, PH)
    av[:, :, 1:57, 1:57] = s
    a1h = ap8.reshape(NCORES, BPC, 2, 128, A1W)
    return [
        {"xprep": np.ascontiguousarray(x_prep[c]),
         "a1h": np.ascontiguousarray(a1h[c]),
         "w3f": W3f, "w1f": W1f, "consts": consts}
        for c in range(NCORES)
    ]


def assemble_out(results):
    outs = [results[c]["out"].reshape(BPC, C, H, W) for c in range(NCORES)]
    return np.ascontiguousarray(
        np.concatenate(outs, axis=0).astype(np.float32)
    )


def _fallback_numpy(x, w3, w1, b11, b12, b13, b21, b22, b23,
                    g1, be1, m1, v1, g2, be2, m2, v2, a1, a2):
    # Straightforward reference math in numpy; only used if an assumption of
    # the device kernel (a1 > 0, finite folded consts) is violated.
    def cb(p):
        return p[None, :, None, None]

    def conv_np(a, w, pad):
        N, Ci, Hh, Ww = a.shape
        O, I, kh, kw = w.shape
        ap = np.pad(a, ((0, 0), (0, 0), (pad, pad), (pad, pad)))
        out = np.zeros((N, O, Hh, Ww), np.float32)
        wm = w.reshape(O, -1)
        for n in range(N):
            cols = np.empty((I * kh * kw, Hh * Ww), np.float32)
            idx = 0
            for i in range(I):
                for dh in range(kh):
                    for dw in range(kw):
                        cols[idx] = ap[n, i, dh : dh + Hh, dw : dw + Ww].ravel()
                        idx += 1
            out[n] = (wm @ cols).reshape(O, Hh, Ww)
        return out

    def bn(t, g, b, mm, v):
        inv = g / np.sqrt(v + EPS)
        return t * cb(inv) + cb(b - mm * inv)

    def prelu(t, a):
        return np.where(t > 0, t, cb(a) * t)

    s3 = np.mean(np.abs(w3), axis=(1, 2, 3), keepdims=True)
    s1 = np.mean(np.abs(w1), axis=(1, 2, 3), keepdims=True)
    o1 = conv_np(np.sign(x + cb(b11)), np.sign(w3) * s3, 1)
    o1 = x + bn(o1, g1, be1, m1, v1)
    o1 = prelu(o1 + cb(b12), a1) + cb(b13)
    o2 = conv_np(np.sign(o1 + cb(b21)), np.sign(w1) * s1, 0)
    o2 = bn(o2, g2, be2, m2, v2) + o1
    o2 = prelu(o2 + cb(b22), a2) + cb(b23)
    return o2.astype(np.float32)


def kernel(**inputs):
    inputs = {k: np.asarray(v) for k, v in inputs.items()}
    in_maps = make_in_maps(**inputs)
    if not _FLAGS.get("ok", True):
        return _fallback_numpy(**{k: np.asarray(v, np.float32)
                                  for k, v in inputs.items()})
    res = _run(in_maps, trace=False)
    return assemble_out(res.results)


# revision 11
# speedup vs baseline: 1.1027x; 1.0662x over previous
"""Trainium2 Bass kernel for the binary-conv BasicBlock (dense_cnn).

Computation (forward values only):
  A1   = sign(x + b11)
  out1 = x + bn1(conv3x3(A1, binw(w3)))          binw(w) = mean|w| * sign(w)
  o1   = prelu(out1 + b12, a1) + b13
  A2   = sign(o1 + b21)
  out2 = bn2(conv1x1(A2, binw(w1))) + o1
  out  = prelu(out2 + b22, a2) + b23

Strategy: data-parallel over the batch axis, 4 images per NeuronCore on 8
cores; weights/consts replicated.  Per core the 3x3 binary conv runs as 9
shifted DoubleRow fp8 matmuls over 464-column tiles spanning padded rows
1..56 only (3248 cols).  A1 = sign(x + b11) is a pure function of the
inputs, so it is packed host-side (fp8, zero halo baked in) and DMAed —
no on-device prep at all.  Element-wise work is then split DVE/ACT so
the TensorE (~27.6us/image) is the only bottleneck (GpSimd's software
tensor ops measure ~10x below roofline, so it only gets the rare b23 op):
  DVE:    (a) t = psum1*sh1 + xprep      (d) u = psum2*sh2 + p1   [in-place]
  ACT:    (b) p1 = prelu(t, a1), (c) a2 = Sign(t - tau),
          (e) out = prelu(u + K2', a2)  [contiguous; host does compaction]
a2 = sign(t - tau) replaces sign(prelu(t,a1)+beta) exactly for a1 > 0
(tau = -beta/a1 if beta >= 0 else -beta).  conv2 matmuls of image i are
interleaved into image i+1's conv1 PE stream via a pending queue so the
PE never idles on drains; image 3's tail is drained at quarter-image
granularity.  Weights load on the ACT DMA ring (the Sync ring starts on
img0 activations at once); outputs share the Sync HW ring — the GpSimd
ring is software-DGE and far too slow for bulk traffic.
"""

import numpy as np
import ml_dtypes
from collections import deque

C = 256
H = W = 56
PH = 58                    # padded image side
NPIX = PH * PH             # 3364
TW = 464                   # matmul tile width (8 padded rows)
NT = 7                     # tiles per image
TOT = NT * TW              # 3248 = 56 rows * 58 cols  (padded rows 1..56)
G0 = PH                    # first computed pixel (row 1, col 0) in A1 coords
HALO = 16
A1W = 3408                 # 16 + NPIX + 16 -> next multiple of 16
BPC = 4                    # images per core
NCORES = 8
EPS = 1e-5

_CACHE = {}
_FLAGS = {"has_b23": False}


def _split_drain_waits(m, max_waits=1):
    """This toolchain's walrus rejects instructions carrying more than ~1-2
    sync waits; hoist extra waits onto preceding single-wait EventSemaphore
    ops on the same engine (semantically identical: the engine blocks on
    each wait in sequence before executing the instruction)."""
    import copy as _copy
    from concourse import mybir

    new_module = _copy.replace(m, functions=[])
    for function in m.functions:
        new_function = _copy.replace(function, blocks=[])
        new_function.set_allocations_from_list(function.allocations)
        for block in function.blocks:
            out = []
            for inst in block.instructions:
                si = inst.sync_info
                if si is not None and len(si.on_wait) > max_waits:
                    waits = list(si.on_wait)
                    keep = waits[:max_waits] if not isinstance(
                        inst, mybir.InstDrain) else []
                    hoist = waits[len(keep):]
                    for i, wt in enumerate(hoist):
                        out.append(
                            mybir.InstEventSemaphore(
                                name=f"{inst.name}-wsplit{i}",
                                opcode="EventSemaphore",
                                engine=inst.engine,
                                sync_info=mybir.SyncInfo(on_wait=[wt], on_update=[]),
                            )
                        )
                    inst.sync_info = mybir.SyncInfo(
                        on_wait=keep, on_update=list(si.on_update)
                    )
                out.append(inst)
            new_block = _copy.replace(block, instructions=out)
            new_function.blocks.append(new_block)
        new_module.functions.append(new_function)
    return new_module


def build_nc(has_b23=None):
    """Build (once per structure flag) the per-core Bass program."""
    if has_b23 is None:
        has_b23 = _FLAGS["has_b23"]
    key = ("nc", has_b23)
    if key in _CACHE:
        return _CACHE[key]
    import concourse.bass as bass
    import concourse.tile as tile
    from concourse import mybir

    Alu = mybir.AluOpType
    AF = mybir.ActivationFunctionType
    f32 = mybir.dt.float32
    bf16 = mybir.dt.bfloat16
    fp8 = mybir.dt.float8e4
    DR = mybir.MatmulPerfMode.DoubleRow

    nc = bass.Bass(trn_type="TRN2", debug=False)
    # xprep = x + K1, stored for padded rows 1..56 (58 cols each, col 0/57
    # are zero pads): dram index = (row-1)*58 + col
    x_d = nc.dram_tensor("xprep", [BPC, 2, 128, TOT], f32, kind="ExternalInput")
    # A1 = sign(x + b11), host-packed fp8 in halo layout (16 + 3364 + 16)
    a_d = nc.dram_tensor("a1h", [BPC, 2, 128, A1W], fp8, kind="ExternalInput")
    w3_d = nc.dram_tensor("w3f", [128, 9 * 2 * 2 * 128], fp8, kind="ExternalInput")
    w1_d = nc.dram_tensor("w1f", [128, 2 * 2 * 128], fp8, kind="ExternalInput")
    c_d = nc.dram_tensor("consts", [2, 128, 8], f32, kind="ExternalInput")
    # padded output rows (58 cols, col 0/57 junk) — compacted host-side
    o_d = nc.dram_tensor("out", [BPC, 2, 128, TOT], f32, kind="ExternalOutput")

    with tile.TileContext(nc) as tc:
        with (
            tc.tile_pool(name="wpool", bufs=1) as wpool,
            tc.tile_pool(name="xpool", bufs=2) as xpool,
            tc.tile_pool(name="a1pool", bufs=2) as a1pool,
            tc.tile_pool(name="tpool", bufs=2) as tpool,
            tc.tile_pool(name="p1pool", bufs=2) as p1pool,
            tc.tile_pool(name="a2pool", bufs=2) as a2pool,
            tc.tile_pool(name="opool", bufs=2) as opool,
            tc.tile_pool(name="ps1", bufs=4, space="PSUM") as ps1p,
            tc.tile_pool(name="ps2", bufs=4, space="PSUM") as ps2p,
        ):
            # ---- constants / weights (resident); weights ride the Vector
            # DMA ring so the Sync ring starts on img0 activations at once
            csb = []
            for kc in range(2):
                ct = wpool.tile([128, 8], f32, tag=f"c_{kc}")
                nc.sync.dma_start(ct[:], c_d.ap()[kc])
                csb.append(ct)
            w3sb = wpool.tile([128, 9 * 2 * 2 * 128], fp8, tag="w3")
            nc.scalar.dma_start(w3sb[:], w3_d.ap())
            w1sb = wpool.tile([128, 2 * 2 * 128], fp8, tag="w1")
            nc.scalar.dma_start(w1sb[:], w1_d.ap())
            w3v = w3sb[:].rearrange("p (g two m) -> p g two m", two=2, m=128)
            w1v = w1sb[:].rearrange("p (g two m) -> p g two m", two=2, m=128)

            def cc(kc, j):
                return csb[kc][:, j : j + 1]

            # const j-layout: 0 unused, 1 -tau, 2 K2', 3 a1, 4 a2, 5 b23,
            # 6 sh1, 7 sh2

            # ---- per-image state ----
            xm = [None] * BPC   # xprep tiles: [kc]
            a1m = [None] * BPC  # A1 tiles (host-packed)
            tbs = [None] * BPC  # t tiles per mc (bf16)
            pbs = [None] * BPC  # p1/u tiles per mc (bf16)
            a2s = [None] * BPC  # a2 tiles [128, 2, TOT] fp8
            outs = [None] * BPC  # padded out tiles [128, TOT] f32 per mc
            pending = deque()   # conv2 closures: one MM2 + one (d) each

            def alloc_img(img):
                xm[img] = [
                    xpool.tile([128, TOT], f32, tag=f"xk_{kc}", name=f"xk_{kc}")
                    for kc in range(2)
                ]
                a1m[img] = a1pool.tile([128, 2 * A1W], fp8, tag="a1m", name="a1m")

            def dma_a1(img):
                for kc in range(2):
                    nc.sync.dma_start(
                        a1m[img][:, kc * A1W : (kc + 1) * A1W], a_d.ap()[img, kc]
                    )

            def dma_x(img, kc, chunks=1):
                n = TOT // chunks
                for ci in range(chunks):
                    nc.sync.dma_start(
                        xm[img][kc][:, ci * n : (ci + 1) * n],
                        x_d.ap()[img, kc][:, ci * n : (ci + 1) * n],
                    )

            def a1_rhs(img, t, kh, kw):
                v = a1m[img][:].rearrange("p (two w) -> p two w", two=2)
                off = HALO + G0 + TW * t + (kh - 1) * PH + (kw - 1)
                return v[:, :, off : off + TW]

            def emit_b(img, lo, hi):
                # p1 = prelu(t, a1) on ACT
                for mc in range(2):
                    nc.scalar.activation(
                        pbs[img][mc][:, lo:hi], tbs[img][mc][:, lo:hi],
                        AF.Prelu, alpha=cc(mc, 3),
                    )

            def emit_c(img, lo, hi):
                # a2 = Sign(t - tau) on ACT (bias slot carries -tau)
                for mc in range(2):
                    nc.scalar.activation(
                        a2s[img][:, mc, lo:hi], tbs[img][mc][:, lo:hi],
                        AF.Sign, bias=cc(mc, 1),
                    )

            def emit_eg(img, h0, h1):
                # (e) out = prelu(u + K2', a2), contiguous in padded coords;
                # the output DMA compacts 58 -> 56 cols (on the GpSimd ring)
                for mc in range(2):
                    ot = outs[img][mc]
                    nc.scalar.activation(
                        ot[:, h0 * PH : h1 * PH],
                        pbs[img][mc][:, h0 * PH : h1 * PH],
                        AF.Prelu, bias=cc(mc, 2), alpha=cc(mc, 4),
                    )
                    if has_b23:
                        nc.gpsimd.tensor_scalar(
                            ot[:, h0 * PH : h1 * PH], ot[:, h0 * PH : h1 * PH],
                            cc(mc, 5), None, Alu.add,
                        )
                    nc.sync.dma_start(
                        o_d.ap()[img, mc][:, h0 * PH : h1 * PH],
                        ot[:, h0 * PH : h1 * PH],
                    )

            def queue_conv2(img):
                outs[img] = [
                    opool.tile([128, TOT], f32, tag=f"o_{mc}", name=f"o_{mc}")
                    for mc in range(2)
                ]

                def mk(t, mc):
                    def emit():
                        ps = ps2p.tile([128, 512], f32, tag="ps2", name="ps2")
                        nc.tensor.matmul(
                            ps[:, :TW], w1v[:, mc],
                            a2s[img][:, :, TW * t : TW * (t + 1)],
                            start=True, stop=True, perf_mode=DR,
                        )
                        sl = pbs[img][mc][:, TW * t : TW * (t + 1)]
                        # (d): u = psum2*sh2 + p1, in place over p1
                        nc.vector.scalar_tensor_tensor(
                            sl, ps[:, :TW], cc(mc, 7), sl, Alu.mult, Alu.add
                        )
                    return emit

                for t in range(NT):
                    for mc in range(2):
                        pending.append(mk(t, mc))

            def conv1(img):
                last = img == BPC - 1
                tbs[img] = [
                    tpool.tile([128, TOT], bf16, tag=f"t_{mc}", name=f"t_{mc}")
                    for mc in range(2)
                ]
                pbs[img] = [
                    p1pool.tile([128, TOT], bf16, tag=f"p1_{mc}", name=f"p1_{mc}")
                    for mc in range(2)
                ]
                a2s[img] = a2pool.tile([128, 2, TOT], fp8, tag="a2", name="a2")
                for t in range(NT):
                    for mc in range(2):
                        ps = ps1p.tile([128, 512], f32, tag="ps1", name="ps1")
                        for sh in range(9):
                            kh, kw = divmod(sh, 3)
                            nc.tensor.matmul(
                                ps[:, :TW], w3v[:, sh * 2 + mc],
                                a1_rhs(img, t, kh, kw),
                                start=(sh == 0), stop=(sh == 8), perf_mode=DR,
                            )
                        # (a): t = psum1*sh1 + xprep
                        nc.vector.scalar_tensor_tensor(
                            tbs[img][mc][:, TW * t : TW * (t + 1)],
                            ps[:, :TW], cc(mc, 6),
                            xm[img][mc][:, TW * t : TW * (t + 1)],
                            Alu.mult, Alu.add,
                        )
                        if pending:
                            pending.popleft()()
                    # hooks
                    if t == 0 and not last:
                        alloc_img(img + 1)
                        dma_a1(img + 1)
                        dma_x(img + 1, 0)
                    elif t == 1 and last:
                        emit_c(img, 0, 2 * TW)
                    elif t == 2 and not last:
                        dma_x(img + 1, 1)
                    elif t == 3:
                        if last:
                            emit_c(img, 2 * TW, 4 * TW)
                        else:
                            emit_c(img, 0, 4 * TW)
                        emit_b(img, 0, 4 * TW)
                    elif t == 4 and img > 0:
                        emit_eg(img - 1, 0, 28)
                    elif t == 5 and last:
                        emit_c(img, 4 * TW, 6 * TW)
                # post-loop
                if last:
                    emit_c(img, 6 * TW, TOT)
                else:
                    emit_c(img, 4 * TW, TOT)
                emit_b(img, 4 * TW, TOT)
                if img > 0:
                    emit_eg(img - 1, 28, 56)

            # ---- main sequence ----
            alloc_img(0)
            dma_a1(0)
            dma_x(0, 0, chunks=4)
            dma_x(0, 1, chunks=4)
            for img in range(BPC):
                conv1(img)
                queue_conv2(img)
            # tail: drain image 3's conv2 at quarter-image granularity
            li = BPC - 1
            for qi, (h0, h1) in enumerate(((0, 14), (14, 28), (28, 42), (42, 56))):
                npop = 4 if qi < 3 else 2
                for _ in range(npop):
                    if pending:
                        pending.popleft()()
                emit_eg(li, h0, h1)

    _CACHE[key] = nc
    return nc


def _host_fold(w3, w1, b11, b12, b13, b21, b22, b23,
               g1, be1, m1, v1, g2, be2, m2, v2, a1, a2):
    f = np.float32
    s3 = np.mean(np.abs(w3), axis=(1, 2, 3)).astype(f)
    s1 = np.mean(np.abs(w1), axis=(1, 2, 3)).astype(f)
    inv1 = (g1 / np.sqrt(v1 + EPS)).astype(f)
    inv2 = (g2 / np.sqrt(v2 + EPS)).astype(f)
    sh1 = s3 * inv1
    ch1 = be1 - m1 * inv1
    sh2 = s1 * inv2
    ch2 = be2 - m2 * inv2
    K1 = (ch1 + b12).astype(f)
    beta = (b13 + b21).astype(f)
    # a2 = sign(prelu(t, a1) + beta) = sign(t - tau) for a1 > 0
    tau = np.where(beta >= 0, -beta / a1, -beta).astype(f)
    K2p = (ch2 + b13 + b22).astype(f)

    fp8 = ml_dtypes.float8_e4m3
    # DoubleRow lhsT layout: [k, ((sh*2+mc)*2+i)*128+m] with i the K-half
    W3 = np.sign(w3).astype(fp8)                                # [O, I, 3, 3]
    W3 = W3.reshape(2, 128, 2, 128, 3, 3)                       # [mc, m, i, k, kh, kw]
    W3 = W3.transpose(3, 4, 5, 0, 2, 1)                         # [k, kh, kw, mc, i, m]
    W3f = np.ascontiguousarray(W3.reshape(128, 9 * 2 * 2 * 128))
    W1 = np.sign(w1).astype(fp8)                                # [O, I, 1, 1]
    W1 = W1.reshape(2, 128, 2, 128)                             # [mc, m, i, k]
    W1 = W1.transpose(3, 0, 2, 1)                               # [k, mc, i, m]
    W1f = np.ascontiguousarray(W1.reshape(128, 2 * 2 * 128))

    consts = np.zeros((2, 128, 8), f)
    for kc in range(2):
        sl = slice(kc * 128, (kc + 1) * 128)
        consts[kc, :, 1] = -tau[sl]
        consts[kc, :, 2] = K2p[sl]
        consts[kc, :, 3] = a1[sl]
        consts[kc, :, 4] = a2[sl]
        consts[kc, :, 5] = b23[sl]
        consts[kc, :, 6] = sh1[sl]
        consts[kc, :, 7] = sh2[sl]
    ok = bool((a1 > 0).all()) and bool(np.isfinite(consts).all())
    return W3f, W1f, consts, K1, ok, bool(np.any(b23 != 0))


def _run(in_maps, trace=False, tmpdir=None, trace_kwargs={}):
    from concourse import bass_utils

    nc = build_nc()
    skey = ("split", _FLAGS["has_b23"])
    if not _CACHE.get(skey):
        # walrus workaround applied only for the HW path (CoreSim rejects
        # post-scheduling instruction edits)
        nc.m = _split_drain_waits(nc.m)
        _CACHE[skey] = True
    return bass_utils.run_bass_kernel_spmd(
        nc,
        in_maps,
        core_ids=list(range(NCORES)),
        trace=trace,
        tmpdir=tmpdir,
        trace_kwargs=trace_kwargs,
    )


def make_in_maps(x, w3, w1, **params):
    x = np.asarray(x, np.float32)
    b11 = np.asarray(params["b11"], np.float32)
    W3f, W1f, consts, K1, ok, has_b23 = _host_fold(
        np.asarray(w3, np.float32), np.asarray(w1, np.float32),
        **{k: np.asarray(v, np.float32) for k, v in params.items()})
    _FLAGS["has_b23"] = has_b23
    _FLAGS["ok"] = ok
    xp = np.zeros((x.shape[0], C, H, PH), np.float32)
    xp[:, :, :, 1:57] = x + K1[None, :, None, None]
    x_prep = xp.reshape(NCORES, BPC, 2, 128, TOT)
    # host-packed A1 = sign(x + b11), fp8, zero halo/pads baked in
    ap8 = np.zeros((x.shape[0], C, A1W), ml_dtypes.float8_e4m3)
    s = np.sign(x + b11[None, :, None, None]).astype(ml_dtypes.float8_e4m3)
    av = ap8[:, :, HALO : HALO + NPIX].reshape(x.shape[0], C, PH, PH)
    av[:, :, 1:57, 1:57] = s
    a1h = ap8.reshape(NCORES, BPC, 2, 128, A1W)
    return [
        {"xprep": np.ascontiguousarray(x_prep[c]),
         "a1h": np.ascontiguousarray(a1h[c]),
         "w3f": W3f, "w1f": W1f, "consts": consts}
        for c in range(NCORES)
    ]


def assemble_out(results):
    outs = [
        results[c]["out"].reshape(BPC, C, H, PH)[:, :, :, 1:57]
        for c in range(NCORES)
    ]
    return np.ascontiguousarray(
        np.concatenate(outs, axis=0).astype(np.float32)
    )


def _fallback_numpy(x, w3, w1, b11, b12, b13, b21, b22, b23,
                    g1, be1, m1, v1, g2, be2, m2, v2, a1, a2):
    # Straightforward reference math in numpy; only used if an assumption of
    # the device kernel (a1 > 0, finite folded consts) is violated.
    def cb(p):
        return p[None, :, None, None]

    def conv_np(a, w, pad):
        N, Ci, Hh, Ww = a.shape
        O, I, kh, kw = w.shape
        ap = np.pad(a, ((0, 0), (0, 0), (pad, pad), (pad, pad)))
        out = np.zeros((N, O, Hh, Ww), np.float32)
        wm = w.reshape(O, -1)
        for n in range(N):
            cols = np.empty((I * kh * kw, Hh * Ww), np.float32)
            idx = 0
            for i in range(I):
                for dh in range(kh):
                    for dw in range(kw):
                        cols[idx] = ap[n, i, dh : dh + Hh, dw : dw + Ww].ravel()
                        idx += 1
            out[n] = (wm @ cols).reshape(O, Hh, Ww)
        return out

    def bn(t, g, b, mm, v):
        inv = g / np.sqrt(v + EPS)
        return t * cb(inv) + cb(b - mm * inv)

    def prelu(t, a):
        return np.where(t > 0, t, cb(a) * t)

    s3 = np.mean(np.abs(w3), axis=(1, 2, 3), keepdims=True)
    s1 = np.mean(np.abs(w1), axis=(1, 2, 3), keepdims=True)
    o1 = conv_np(np.sign(x + cb(b11)), np.sign(w3) * s3, 1)
    o1 = x + bn(o1, g1, be1, m1, v1)
    o1 = prelu(o1 + cb(b12), a1) + cb(b13)
    o2 = conv_np(np.sign(o1 + cb(b21)), np.sign(w1) * s1, 0)
    o2 = bn(o2, g2, be2, m2, v2) + o1
    o2 = prelu(o2 + cb(b22), a2) + cb(b23)
    return o2.astype(np.float32)


def kernel(**inputs):
    inputs = {k: np.asarray(v) for k, v in inputs.items()}
    in_maps = make_in_maps(**inputs)
    if not _FLAGS.get("ok", True):
        return _fallback_numpy(**{k: np.asarray(v, np.float32)
                                  for k, v in inputs.items()})
    res = _run(in_maps, trace=False)
    return assemble_out(res.results)
